# revision 35
# baseline (speedup 1.0000x reference)
"""Cross-attention kernel for Trainium2 (8 NeuronCores, SPMD).

Problem: B=4, Nq=1024, Nk=2048, D=512, 8 heads x 64 head-dim, fp32,
full-tensor bias added to scores before softmax.

Sharding: (batch, head-half) -> 8 shards. Core 2b+hh computes heads
[4hh, 4hh+4) of batch b over ALL 1024 queries. K/V projections are not
duplicated (each core projects only its 4 heads). The output projection
is computed per head-half (row-sharded Wo) and the two partial [512,1024]
results of a batch are summed on the host (plus bo), which replaces the
on-device all-reduce.

Device layout: attention tensors transposed (feature/key dim on
partitions) so every contraction lands on the partition axis:
  QT[d', q] = (SCALE*Wq_hh) @ xT       KT[d', k] = Wk_hh @ ctxT
  V[k, i]   = ctxT.T @ Wv_hh.T
  ST[k, q]  = KT_h.T @ QT_h            (contraction over 64 head dims; the
                                        two heads of a pair sit in PE row
                                        groups 0-1/2-3 and run concurrently)
  E = exp(ST) * exp(biasT - 4)         (ACT exp on a 2-bank PSUM pair tile;
                                        host sends exp(bias-4).T so the add
                                        becomes a multiply; -4 cancels in
                                        the softmax normalization)
  o2[i(+1), q] = [V_h | 1].T @ E       (ones column gives softmax row sums)
  OT = o2[0:64] * recip(sum)           (sums DMA-reshaped to [128,8] for a
                                        parallel reciprocal, DMA-replicated
                                        back across partitions)
  yT[d, q] = Wo_hh @ OT                (partial; host sums the two halves)
Queries processed in two 512-column sections so each o2 accumulator is one
PSUM bank. The exp chain is the pacer (~1.15us per 128-key chunk); K/Q/V
projections for the second pair and the section-0 output projection are
interleaved into the attention loops as TensorE gap filler, and scores for
chunk c+1 are issued before o2 of chunk c so the PE never sits behind the
ACT->DVE dependency. Matmul operands fp16 (fp32 PSUM accumulate).
"""

import numpy as np
import concourse.bass as bass
import concourse.bacc as bacc
import concourse.mybir as mybir
import concourse.tile as tile
from concourse import bass_utils

HEADS = 8
DH = 64
D = 512
NH = 4            # heads per core
INNER = NH * DH   # 256
NQ = 1024         # all queries on every core
NS = 512          # query section width
NK = 2048
KC = NK // 128    # 16 key chunks
SCALE = DH ** -0.5
BSHIFT = 4.0      # exp(bias - BSHIFT): keeps fp16 weights in range

F32 = mybir.dt.float32
F16 = mybir.dt.float16
AF = mybir.ActivationFunctionType


def _bcast2(ap, n):
    """[128, F] -> [128, n, F] with a step-0 middle dim."""
    return bass.AP(ap.tensor, ap.offset, [ap.ap[0], [0, n], ap.ap[1]])


def _build_nc():
    nc = bacc.Bacc("TRN2", target_bir_lowering=False, debug=False)

    xT_d = nc.dram_tensor("xT", [D, NQ], F16, kind="ExternalInput")
    ctxT_d = nc.dram_tensor("ctxT", [D, NK], F16, kind="ExternalInput")
    expB_d = nc.dram_tensor("expB", [NK, NQ], F16, kind="ExternalInput")
    wqT_d = nc.dram_tensor("wqT", [D, INNER], F16, kind="ExternalInput")
    wkT_d = nc.dram_tensor("wkT", [D, INNER], F16, kind="ExternalInput")
    wvT_d = nc.dram_tensor("wvT", [D, INNER], F16, kind="ExternalInput")
    woT_d = nc.dram_tensor("woT", [INNER, D], F16, kind="ExternalInput")
    yT_d = nc.dram_tensor("yT", [D, NQ], F16, kind="ExternalOutput")
    scr_d = [nc.dram_tensor(f"scr{i}", [2 * NS], F16) for i in range(4)]

    with tile.TileContext(nc) as tc, nc.allow_low_precision(
            reason="fp16 matmul operands, fp32 accumulation"):
        with (
            tc.tile_pool(name="const", bufs=1) as const,
            tc.tile_pool(name="main", bufs=1) as main,
            tc.tile_pool(name="work", bufs=5) as work,
            tc.tile_pool(name="nrp", bufs=2) as nrp,
            tc.tile_pool(name="psS", bufs=2, space="PSUM") as psS,
            tc.tile_pool(name="psO", bufs=2, space="PSUM") as psO,
            tc.tile_pool(name="psA", bufs=2, space="PSUM") as psA,
        ):
            # ---- DMA loads: per-tile full-width transfers -- each is one
            # LINEAR 64-512KB DRAM read (interleaved/packed layouts break
            # into thousands of small pieces and are far slower) ----
            wq = [const.tile([128, INNER], F16, name=f"wq{i}", tag=f"wq{i}") for i in range(4)]
            wk = [const.tile([128, INNER], F16, name=f"wk{i}", tag=f"wk{i}") for i in range(4)]
            wv = [const.tile([128, INNER], F16, name=f"wv{i}", tag=f"wv{i}") for i in range(4)]
            wo = [const.tile([128, D], F16, name=f"wo{i}", tag=f"wo{i}") for i in range(2)]
            xts = [const.tile([128, NQ], F16, name=f"xts{i}", tag=f"xts{i}") for i in range(4)]
            ctx = [const.tile([128, NK], F16, name=f"ctx{i}", tag=f"ctx{i}") for i in range(4)]
            eB = [main.tile([128, NQ], F16, name=f"eB{c}", tag=f"eB{c}") for c in range(KC)]
            for i in range(4):
                sl = slice(i * 128, (i + 1) * 128)
                nc.sync.dma_start(out=wq[i], in_=wqT_d[sl, :])
                nc.scalar.dma_start(out=wk[i], in_=wkT_d[sl, :])
                nc.gpsimd.dma_start(out=ctx[i], in_=ctxT_d[sl, :])
            for i in range(4):
                sl = slice(i * 128, (i + 1) * 128)
                nc.sync.dma_start(out=xts[i], in_=xT_d[sl, :])
                nc.scalar.dma_start(out=wv[i], in_=wvT_d[sl, :])
            for i in range(2):
                sl = slice(i * 128, (i + 1) * 128)
                nc.gpsimd.dma_start(out=wo[i], in_=woT_d[sl, :])
            for c in range(4):
                nc.gpsimd.dma_start(out=eB[c], in_=expB_d[c * 128:(c + 1) * 128, :])

            KT = [main.tile([128, NK], F16, name=f"KT{p}", tag=f"KT{p}") for p in range(2)]
            QT = [main.tile([128, NQ], F16, name=f"QT{p}", tag=f"QT{p}") for p in range(2)]
            OT = [main.tile([128, NQ], F16, name=f"OT{p}", tag=f"OT{p}") for p in range(2)]
            Vo = [main.tile([128, NH, DH + 1], F16, name=f"Vo{c}", tag=f"Vo{c}")
                  for c in range(KC)]

            onesF = const.tile([128, 1], F32, name="onesF", tag="onesF")
            nc.vector.memset(onesF, 1.0)
            onesK1 = const.tile([1, 128], F16, name="onesK1", tag="onesK1")
            nc.vector.memset(onesK1, 1.0)
            for c in range(KC):
                nc.vector.tensor_copy(
                    Vo[c][:, :, DH], onesF[:, 0:1].broadcast_to([128, NH]))

            # warm-up: ACT table load + junk matmuls to lift the PE HAM
            # clock gate while the first DMAs land
            dumA = const.tile([128, 64], F16, name="dumA", tag="dumA")
            dumB = const.tile([128, 512], F16, name="dumB", tag="dumB")
            dume = const.tile([1, 8], F32, name="dume", tag="dume")
            nc.vector.memset(dumA, 0.0)
            nc.vector.memset(dumB, 0.0)
            nc.vector.memset(dume, 0.0)
            # prime the natural_log_exp table set (covers both Ln and Exp)
            # so the tail's Ln never triggers a mid-kernel table switch
            dumx = const.tile([1, 8], F32, name="dumx", tag="dumx")
            dumy = const.tile([1, 8], F16, name="dumy", tag="dumy")
            nc.vector.memset(dumx, 1.0)
            nc.scalar.activation(dumx, dumx, AF.Ln)
            nc.scalar.activation(dumy, dumx, AF.Exp)
            def warm_mm(n=1):
                for _ in range(n):
                    ps = psA.tile([64, 512], F32, name="warm", tag="proj")
                    nc.tensor.matmul(ps, dumA, dumB, start=True, stop=True)

            warm_mm(10)

            # ---- projection helpers ----
            def q_proj(p, sct):
                ssl = slice(sct * NS, (sct + 1) * NS)
                msl = slice(p * 128, (p + 1) * 128)
                ps = psA.tile([128, NS], F32, name="qproj", tag="proj")
                for ki in range(4):
                    nc.tensor.matmul(ps, wq[ki][:, msl], xts[ki][:, ssl],
                                     start=(ki == 0), stop=(ki == 3))
                nc.vector.tensor_copy(QT[p][:, ssl], ps)

            def k_proj(p, nt):
                nsl = slice(nt * 512, (nt + 1) * 512)
                msl = slice(p * 128, (p + 1) * 128)
                ps = psA.tile([128, 512], F32, name="kproj", tag="proj")
                for ki in range(4):
                    nc.tensor.matmul(ps, wk[ki][:, msl], ctx[ki][:, nsl],
                                     start=(ki == 0), stop=(ki == 3))
                nc.vector.tensor_copy(KT[p][:, nsl], ps)

            def v_proj(c):
                csl = slice(c * 128, (c + 1) * 128)
                ps = psA.tile([128, INNER], F32, name="vproj", tag="proj")
                for ki in range(4):
                    nc.tensor.matmul(ps, ctx[ki][:, csl], wv[ki],
                                     start=(ki == 0), stop=(ki == 3))
                nc.vector.tensor_copy(
                    Vo[c][:, :, 0:DH],
                    ps.rearrange("p (h d) -> p h d", h=NH))

            # output staging: casts land in a packed tile per section,
            # shipped per 128-row block on alternating queues
            ysbF = [const.tile([128, 4, NS], F16, name=f"ysbF{s}", tag=f"ysbF{s}")
                    for s in range(2)]

            def out_proj(mi, sct):
                msl = slice(mi * 128, (mi + 1) * 128)
                ssl = slice(sct * NS, (sct + 1) * NS)
                ps = psA.tile([128, NS], F32, name="oproj", tag="proj")
                for ki in range(2):
                    nc.tensor.matmul(ps, wo[ki][:, msl], OT[ki][:, ssl],
                                     start=(ki == 0), stop=(ki == 1))
                nc.vector.tensor_copy(ysbF[sct][:, mi, :], ps)

            def ship_y(sct, mi):
                q = nc.sync if mi % 2 == 0 else nc.gpsimd
                q.dma_start(out=yT_d[mi * 128:(mi + 1) * 128,
                                     sct * NS:(sct + 1) * NS],
                            in_=ysbF[sct][:, mi, :])

            def norm(sct, p, o2a, o2b, last=False):
                ssl = slice(sct * NS, (sct + 1) * NS)
                oU = nrp.tile([128, NS], F16, name="oU", tag="oU")
                ss2 = nrp.tile([1, 2 * NS], F32, name="ss2", tag="ss2")
                if last:
                    # tail chain with no DMA round trips: ACT (idle after
                    # the final exp) computes 1/s = exp(-ln s) in place,
                    # then two K=1 matmuls broadcast it across partitions
                    nc.vector.tensor_copy(oU[0:DH, :], o2a[0:DH, :])
                    nc.scalar.copy(oU[DH:128, :], o2b[0:DH, :])
                    nc.scalar.copy(ss2[:, 0:NS], o2a[DH:DH + 1, :])
                    nc.vector.tensor_copy(ss2[:, NS:2 * NS], o2b[DH:DH + 1, :])
                    lnss = nrp.tile([1, 2 * NS], F32, name="lnss", tag="lnss")
                    nc.scalar.activation(lnss, ss2, AF.Ln)
                    rr = nrp.tile([1, 2 * NS], F16, name="rr", tag="rr")
                    nc.scalar.activation(rr, lnss, AF.Exp, scale=-1.0)
                    nrmP = psA.tile([128, NS], F32, name="nrmP", tag="proj")
                    nc.tensor.matmul(nrmP[0:DH, :], onesK1[0:1, 0:DH],
                                     rr[0:1, 0:NS], start=True, stop=True)
                    nc.tensor.matmul(nrmP[DH:128, :], onesK1[0:1, DH:128],
                                     rr[0:1, NS:2 * NS], start=True, stop=True)
                    nc.vector.tensor_mul(OT[p][:, ssl], oU, nrmP)
                    return
                nc.vector.tensor_copy(oU[0:DH, :], o2a[0:DH, :])
                nc.vector.tensor_copy(oU[DH:128, :], o2b[0:DH, :])
                nc.vector.tensor_copy(ss2[:, 0:NS], o2a[DH:DH + 1, :])
                nc.vector.tensor_copy(ss2[:, NS:2 * NS], o2b[DH:DH + 1, :])
                st = nrp.tile([128, 8], F32, name="st", tag="st")
                nc.sync.dma_start(out=st, in_=ss2)
                sr = nrp.tile([128, 8], F16, name="sr", tag="sr")
                nc.vector.reciprocal(sr, st)
                d = scr_d[2 * sct + p]
                nc.sync.dma_start(out=d[:], in_=sr)
                nrm = nrp.tile([128, NS], F16, name="nrm", tag="nrm")
                nc.sync.dma_start(
                    out=nrm,
                    in_=bass.AP(d[:].tensor, 0, [[NS, 2], [0, DH], [1, NS]]))
                nc.vector.tensor_mul(OT[p][:, ssl], oU, nrm)

            # ---- minimal upfront: just what chunk 0 of (sect0, pair0)
            # needs; the rest of pair-0's projections ride as fillers in
            # the first loop iterations ----
            q_proj(0, 0)
            warm_mm(2)
            k_proj(0, 0)
            warm_mm(2)
            v_proj(0)
            v_proj(1)

            # ---- attention: 4 sections of 16 chunks, exp-paced pipeline ----
            # pair-major: pair-1 K/Q projections ride inside pair-0 sections
            iters = [(0, 0), (1, 0), (0, 1), (1, 1)]

            def emit_scores(sec, c):
                sct, p = sec
                ssl = slice(sct * NS, (sct + 1) * NS)
                csl = slice(c * 128, (c + 1) * 128)
                s = psS.tile([128, 2, NS], F32, name="s", tag="s")
                nc.tensor.matmul(s[:, 0, :], KT[p][0:DH, csl], QT[p][0:DH, ssl],
                                 start=True, stop=True)
                nc.tensor.matmul(s[:, 1, :], KT[p][DH:128, csl], QT[p][DH:128, ssl],
                                 start=True, stop=True)
                return s

            # pair-1 K/Q projections, one matmul per chunk to keep the PE
            # load per exp period smooth (psA ring: K in slot A, Q in B)
            fil = {}

            def k_proj_1mm(nt, ki):
                if ki == 0:
                    fil["psK"] = psA.tile([128, 512], F32, name="kp1", tag="proj")
                nsl = slice(nt * 512, (nt + 1) * 512)
                nc.tensor.matmul(fil["psK"], wk[ki][:, 128:256], ctx[ki][:, nsl],
                                 start=(ki == 0), stop=(ki == 3))
                if ki == 3:
                    nc.vector.tensor_copy(KT[1][:, nsl], fil["psK"])

            def q_proj_1mm(sct, ki):
                if ki == 0:
                    fil["psQ"] = psA.tile([128, NS], F32, name="qp1", tag="proj")
                ssl = slice(sct * NS, (sct + 1) * NS)
                nc.tensor.matmul(fil["psQ"], wq[ki][:, 128:256], xts[ki][:, ssl],
                                 start=(ki == 0), stop=(ki == 3))
                if ki == 3:
                    nc.vector.tensor_copy(QT[1][:, ssl], fil["psQ"])

            def fillers(si, c):
                if si == 0:
                    # V chunks must stay two ahead of their o2 consumer;
                    # deferred bias chunks stream on the idle gpsimd queue;
                    # rest of pair-0's K/Q projections in the first iters
                    if c == 0:
                        k_proj(0, 1)
                    elif c == 1:
                        q_proj(0, 1)
                    elif c == 2:
                        k_proj(0, 2)
                    elif c == 3:
                        k_proj(0, 3)
                    if c <= 13:
                        v_proj(c + 2)
                    if c <= 11:
                        nc.gpsimd.dma_start(
                            out=eB[c + 4],
                            in_=expB_d[(c + 4) * 128:(c + 5) * 128, :])
                elif si == 1:
                    # slots: K0 c0-3, Q0 c2-5, K1 c4-7, Q1 c6-9, K2 c8-11,
                    # K3 c12-15 -- psA round-robin stays conflict-free
                    if c <= 3:
                        k_proj_1mm(0, c)
                    elif c <= 7:
                        k_proj_1mm(1, c - 4)
                    elif c <= 11:
                        k_proj_1mm(2, c - 8)
                    else:
                        k_proj_1mm(3, c - 12)
                    if 2 <= c <= 5:
                        q_proj_1mm(0, c - 2)
                    elif 6 <= c <= 9:
                        q_proj_1mm(1, c - 6)
                elif si == 2:
                    # no real filler work: keep the PE HAM clock warm
                    warm_mm(1)
                elif si == 3:
                    if c in (6, 9, 12, 14):
                        mi = (6, 9, 12, 14).index(c)
                        out_proj(mi, 0)
                        ship_y(0, mi)
                    else:
                        warm_mm(1)

            s_cur = emit_scores(iters[0], 0)
            for si, (sct, p) in enumerate(iters):
                o2a = psO.tile([DH + 1, NS], F32, name="o2a", tag="o2")
                o2b = psO.tile([DH + 1, NS], F32, name="o2b", tag="o2")
                for c in range(KC):
                    # scores for the next chunk go first so the PE works
                    # ahead while ACT/DVE process the current one
                    s_nxt = None
                    if c < KC - 1:
                        s_nxt = emit_scores((sct, p), c + 1)
                    elif si < 3:
                        s_nxt = emit_scores(iters[si + 1], 0)
                    e1 = work.tile([128, 2, NS], F16, name="e1", tag="e1")
                    nc.scalar.activation(e1, s_cur, AF.Exp)
                    et = work.tile([128, 2, NS], F16, name="et", tag="et")
                    nc.vector.tensor_mul(
                        et, e1, _bcast2(eB[c][:, sct * NS:(sct + 1) * NS], 2))
                    fillers(si, c)
                    nc.tensor.matmul(o2a, Vo[c][:, 2 * p, :], et[:, 0, :],
                                     start=(c == 0), stop=(c == KC - 1))
                    nc.tensor.matmul(o2b, Vo[c][:, 2 * p + 1, :], et[:, 1, :],
                                     start=(c == 0), stop=(c == KC - 1))
                    s_cur = s_nxt
                norm(sct, p, o2a, o2b, last=(si == 3))

            # ---- remaining output projection (section 1) ----
            for mi in range(4):
                out_proj(mi, 1)
                ship_y(1, mi)

    nc.compile()
    return nc


_NC_CACHE = {}


def _get_nc():
    if "nc" not in _NC_CACHE:
        _NC_CACHE["nc"] = _build_nc()
    return _NC_CACHE["nc"]


def make_in_maps(x, context, bias, Wq, Wk, Wv, Wo, bo):
    x = np.asarray(x, dtype=np.float32)
    context = np.asarray(context, dtype=np.float32)
    bias = np.asarray(bias, dtype=np.float32)
    Wq = np.asarray(Wq, dtype=np.float32)
    Wk = np.asarray(Wk, dtype=np.float32)
    Wv = np.asarray(Wv, dtype=np.float32)
    Wo = np.asarray(Wo, dtype=np.float32)

    wqT = [np.ascontiguousarray(
        (Wq[hh * INNER:(hh + 1) * INNER] * SCALE).T).astype(np.float16)
        for hh in range(2)]
    wkT = [np.ascontiguousarray(
        Wk[hh * INNER:(hh + 1) * INNER].T).astype(np.float16) for hh in range(2)]
    wvT = [np.ascontiguousarray(
        Wv[hh * INNER:(hh + 1) * INNER].T).astype(np.float16) for hh in range(2)]
    woT = [np.ascontiguousarray(
        Wo[:, hh * INNER:(hh + 1) * INNER].T).astype(np.float16) for hh in range(2)]

    xTs, ctxTs, expBs = [], [], []
    for b in range(4):
        xTs.append(np.ascontiguousarray(x[b].T).astype(np.float16))
        ctxTs.append(np.ascontiguousarray(context[b].T).astype(np.float16))
        expBs.append(np.ascontiguousarray(
            np.exp(bias[b] - BSHIFT).T).astype(np.float16))

    in_maps = []
    for core in range(8):
        b, hh = core // 2, core % 2
        in_maps.append({
            "xT": xTs[b], "ctxT": ctxTs[b], "expB": expBs[b],
            "wqT": wqT[hh], "wkT": wkT[hh], "wvT": wvT[hh], "woT": woT[hh],
        })
    return in_maps


def kernel(x, context, bias, Wq, Wk, Wv, Wo, bo):
    nc = _get_nc()
    in_maps = make_in_maps(x, context, bias, Wq, Wk, Wv, Wo, bo)
    res = bass_utils.run_bass_kernel_spmd(
        nc, in_maps, core_ids=list(range(8)), trace=False)

    bo = np.asarray(bo, dtype=np.float32)
    out = np.empty((4, NQ, D), dtype=np.float32)
    for b in range(4):
        yT = (res.results[2 * b]["yT"].astype(np.float32)
              + res.results[2 * b + 1]["yT"].astype(np.float32))
        out[b] = yT.T + bo
    return out
